# revision 16
# baseline (speedup 1.0000x reference)
"""Trainium2 Bass kernel for nn_CCGGenerator (LSTM encoder + attention decoder).

Sharding: data-parallel, batch 128 -> 16 per core across 8 cores.
All weights replicated. Self-contained; everything hardcoded.

Per-core design (B=16):
- Encoder gates computed transposed: gates.T [2048, 16] as 16 PSUM tiles
  [128, 16] packed in one [128, 256] region. Weight tiles reordered as
  [i0 i1 f0 f1 g0 g1 o0 o1 | i2 i3 f2 f3 g2 g3 o2 o3] so each half of the
  hidden dim (A = h-dims 0..255, B = 256..511) is a contiguous 128-col block.
- xg (input projection + bias) is FUSED into the same PSUM accumulation as
  16 extra K=33 matmuls (start=True), eliminating the per-step bias adds.
- tanh(g) = 2*sigmoid(2g) - 1 with g-rows of the weights pre-scaled by 2 ->
  ONE sigmoid over each 128-col half instead of 3 activations.
- Tail split into halves A/B whose ops pipeline against the next step's
  matmuls (k-ordered so MMs needing hA run first); LDWEIGHTS fully overlaps
  MMs on the PE so the matmul burst is ~27ns/MM.
- The whole condition-latent CL lives in SBUF: cl_sb [128, N*64] bf16 with
  col = n*64 + k*16 + b  (k = hidden 128-chunk). No DMA in the recurrence.
- Decoder: 24 steps into hd_sb [128, 4*24*16]; attention per b as before.
"""
import sys
sys.path.insert(0, "/opt/trn_rl_repo")

import numpy as np
import ml_dtypes
from contextlib import ExitStack

import concourse.bass as bass
import concourse.tile as tile
from concourse import bacc, mybir
from concourse.bass_utils import run_bass_kernel_spmd

F32 = mybir.dt.float32
BF16 = mybir.dt.bfloat16
AF = mybir.ActivationFunctionType
OP = mybir.AluOpType
BF = ml_dtypes.bfloat16

NCORES = 8
B = 16          # batch per core
N_STEPS = 1024  # encoder sequence length
SCH_C = 64      # steps per cond chunk
C = 32
H = 512
G = 2048        # 4H
T = 24
KH = 4          # hidden 128-chunks
M16 = 16        # gate-dim 128-chunks

# m' tile order: halves of the hidden dim, [i i f f g g o o] within a half
MPRIME = [0, 1, 4, 5, 8, 9, 12, 13, 2, 3, 6, 7, 10, 11, 14, 15]


def build_program(n_steps=N_STEPS):
    nac = n_steps // 128  # attention n-chunks
    nc = bacc.Bacc("TRN2", target_bir_lowering=False, debug=False,
                   num_devices=NCORES)

    p_cond = nc.declare_dram_parameter("cond_aT", [128, n_steps * B], BF16, isOutput=False)
    p_wih = nc.declare_dram_parameter("wihT_a", [128, G], BF16, isOutput=False)
    p_whh = nc.declare_dram_parameter("whhT", [128, 64 * 128], BF16, isOutput=False)
    p_wcell = nc.declare_dram_parameter("wcellT", [128, 64 * 128], BF16, isOutput=False)
    p_bcell = nc.declare_dram_parameter("bcell_bc", [128, 256], F32, isOutput=False)
    p_wout = nc.declare_dram_parameter("woutT", [128, 8 * C], BF16, isOutput=False)
    p_bout = nc.declare_dram_parameter("bout_bc", [T, C], F32, isOutput=False)
    p_id = nc.declare_dram_parameter("ident", [128, 128], BF16, isOutput=False)
    p_out = nc.declare_dram_parameter("out", [B, T, C], F32, isOutput=True)

    with tile.TileContext(nc) as tc, ExitStack() as ctx:
        const = ctx.enter_context(tc.tile_pool(name="const", bufs=1))

        cl_sb = const.tile([128, n_steps * KH * B], BF16, tag="cl_sb")  # 16 MB
        hd_sb = const.tile([128, KH * T * B], BF16, tag="hd_sb")
        wout_sb = const.tile([128, 8 * C], BF16, tag="wout_sb")
        nc.sync.dma_start(wout_sb[:], p_wout[:])
        bout_sb = const.tile([T, C], F32, tag="bout_sb")
        nc.sync.dma_start(bout_sb[:], p_bout[:])
        id_sb = const.tile([128, 128], BF16, tag="id_sb")
        nc.sync.dma_start(id_sb[:], p_id[:])
        bcell_sb = const.tile([128, 256], F32, tag="bcell_sb")
        nc.sync.dma_start(bcell_sb[:], p_bcell[:])

        cl3 = cl_sb[:].rearrange("p (n k b) -> p n k b", k=KH, b=B)
        hd_v = hd_sb[:].rearrange("p (k t b) -> p k t b", k=KH, t=T)

        # ---------------- encoder ----------------
        # Two PSUM partials in separate banks (full-bank [128,512] tiles):
        #   Ge = X + k0 + k1 per tile (gated by hA of prev step)
        #   Gl = k2 + k3 per tile     (gated by hB of prev step)
        # Within a bank each tile's accumulation group is contiguous
        # (start=True clears has_written for the whole bank).  The early
        # partial is copied to SBUF during the gap, then one DVE add
        # (SBUF + PSUM) feeds the sigmoid.  Step parity rotates which half
        # (A = h-dims 0..255 / B) finishes first, so the late-half penalty
        # alternates.
        with tc.tile_pool(name="enc", bufs=1) as enc_pool, \
             tc.tile_pool(name="cond", bufs=2) as cond_pool, \
             tc.tile_pool(name="ga", bufs=2) as ga_pool, \
             tc.tile_pool(name="st", bufs=2) as st_pool, \
             tc.tile_pool(name="gps", bufs=2, space="PSUM") as gps_pool:
            whh_sb = enc_pool.tile([128, 64 * 128], BF16, tag="whh_sb")
            nc.sync.dma_start(whh_sb[:], p_whh[:])
            wih_sb = enc_pool.tile([128, G], BF16, tag="wih_sb")
            nc.sync.dma_start(wih_sb[:], p_wih[:])
            c_f = enc_pool.tile([128, KH * B], F32, tag="c_f")
            h0 = enc_pool.tile([128, KH * B], BF16, tag="h0")
            nc.any.memset(c_f[:], 0.0)
            nc.any.memset(h0[:], 0.0)

            cond_ch = None
            for n in range(n_steps):
                if n % SCH_C == 0:
                    cond_ch = cond_pool.tile([128, SCH_C * B], BF16, tag="cond_ch")
                    nc.sync.dma_start(cond_ch[:], p_cond[:, bass.ts(n // SCH_C, SCH_C * B)])
                soff = (n % SCH_C) * B
                if n == 0:
                    hin = lambda k: h0[:, bass.ts(k, B)]
                else:
                    hin = lambda k, _n=n: cl_sb[:, (_n - 1) * 64 + k * B:(_n - 1) * 64 + (k + 1) * B]

                par = n % 2          # 0: A first; 1: B first
                halves = (0, 1) if par == 0 else (1, 0)
                # Per-half gates PSUM, one full bank each (bufs=2 -> 4 banks).
                # Each tile's accumulation group [X, ke0, ke1, kl0, kl1] is
                # contiguous within its bank; the sigmoid reads PSUM directly.
                gh = [None, None]
                for hf in (0, 1):
                    gh[hf] = gps_pool.tile([128, 512], F32, tag=f"gh{hf}", name=f"gh{hf}")
                # ke: chunks of the h half that step n-1 finished first (the
                # opposite parity's first half); kl: the other two chunks.
                ke = (2, 3) if par == 0 else (0, 1)
                kl = (0, 1) if par == 0 else (2, 3)
                for hf in halves:
                    for j in range(8):
                        m = hf * 8 + j
                        nc.tensor.matmul(gh[hf][:, bass.ts(j, B)], wih_sb[:, bass.ts(m, 128)],
                                         cond_ch[:, soff:soff + B], start=True, stop=False)
                        for k in (ke[0], ke[1], kl[0], kl[1]):
                            nc.tensor.matmul(gh[hf][:, bass.ts(j, B)],
                                             whh_sb[:, bass.ts(m * KH + k, 128)],
                                             hin(k), start=False, stop=(k == kl[1]))
                # tails: the early half's FULL chain first, then the late half's
                ga = [None, None]
                for hf in halves:
                    hs = hf * 32
                    ga[hf] = ga_pool.tile([128, 128], F32, tag=f"ga{hf}", name=f"ga{hf}")
                    nc.scalar.activation(ga[hf][:], gh[hf][:, 0:128], AF.Sigmoid)
                    g_sl = ga[hf][:, 64:96]
                    nc.vector.tensor_scalar(g_sl, g_sl, 2.0, -1.0, op0=OP.mult, op1=OP.add)
                    c2 = st_pool.tile([128, 32], F32, tag=f"c2{hf}", name="c2")
                    nc.vector.tensor_tensor(c2[:], ga[hf][:, 32:64], c_f[:, hs:hs + 32], op=OP.mult)
                    ig = st_pool.tile([128, 32], F32, tag=f"ig{hf}", name="ig")
                    nc.vector.tensor_tensor(ig[:], ga[hf][:, 0:32], g_sl, op=OP.mult)
                    nc.vector.tensor_tensor(c_f[:, hs:hs + 32], c2[:], ig[:], op=OP.add)
                    th = st_pool.tile([128, 32], F32, tag=f"th{hf}", name="th")
                    nc.scalar.activation(th[:], c_f[:, hs:hs + 32], AF.Tanh)
                    nc.vector.tensor_tensor(cl_sb[:, n * 64 + hs:n * 64 + hs + 32],
                                            ga[hf][:, 96:128], th[:], op=OP.mult)

        # ---------------- decoder (baseline structure) ----------------
        with tc.tile_pool(name="dec", bufs=1) as dec_pool, \
             tc.tile_pool(name="dg_ps", bufs=2, space="PSUM") as dg_ps_pool, \
             tc.tile_pool(name="dgtmp", bufs=2) as dgtmp_pool, \
             tc.tile_pool(name="dst", bufs=2) as dst_pool:
            wcell_sb = dec_pool.tile([128, 64 * 128], BF16, tag="wcell_sb")
            nc.sync.dma_start(wcell_sb[:], p_wcell[:])
            for t in range(T):
                if t == 0:
                    hin = lambda k: cl_sb[:, (n_steps - 1) * 64 + k * B:(n_steps - 1) * 64 + (k + 1) * B]
                else:
                    hin = lambda k, _t=t: hd_sb[:, k * T * B + (_t - 1) * B:k * T * B + _t * B]
                gps = dg_ps_pool.tile([128, 256], F32, tag="gps")
                for m in range(M16):
                    for k in range(KH):
                        nc.tensor.matmul(gps[:, bass.ts(m, B)],
                                         wcell_sb[:, bass.ts(m * KH + k, 128)],
                                         hin(k), start=(k == 0), stop=(k == KH - 1))
                ga = dgtmp_pool.tile([128, 256], F32, tag="ga")
                nc.vector.tensor_tensor(ga[:], gps[:], bcell_sb[:], op=OP.add)
                nc.scalar.activation(ga[:, 0:128], ga[:, 0:128], AF.Sigmoid)     # i, f
                nc.scalar.activation(ga[:, 128:192], ga[:, 128:192], AF.Tanh)    # g
                nc.scalar.activation(ga[:, 192:256], ga[:, 192:256], AF.Sigmoid)  # o
                cd = dst_pool.tile([128, 64], F32, tag="cd")
                nc.vector.tensor_tensor(cd[:], ga[:, 0:64], ga[:, 128:192], op=OP.mult)
                th = dst_pool.tile([128, 64], F32, tag="th")
                nc.scalar.activation(th[:], cd[:], AF.Tanh)
                for k in range(KH):
                    nc.vector.tensor_tensor(hd_sb[:, k * T * B + t * B:k * T * B + (t + 1) * B],
                                            ga[:, 192 + k * B:192 + (k + 1) * B],
                                            th[:, bass.ts(k, B)], op=OP.mult)

        # ---------------- attention + output, per batch ----------------
        with tc.tile_pool(name="att_fix", bufs=2) as att_fix, \
             tc.tile_pool(name="scr_ps", bufs=1, space="PSUM") as scr_ps_pool, \
             tc.tile_pool(name="tp_ps", bufs=2, space="PSUM") as tp_ps_pool, \
             tc.tile_pool(name="ctx_ps", bufs=2, space="PSUM") as ctx_ps_pool:
            for b in range(B):
                # scores [24, n]: lhsT = hd strided, rhs = cl_sb strided (CL.T native)
                scr = scr_ps_pool.tile([T, n_steps], F32, tag="scr")
                scn = min(512, n_steps)
                for k in range(KH):
                    for j in range(n_steps // scn):
                        rhs = cl3[:, j * scn:(j + 1) * scn, k, b]
                        nc.tensor.matmul(scr[:, bass.ts(j, scn)], hd_v[:, k, :, b],
                                         rhs, start=(k == 0), stop=(k == KH - 1))
                nmx = att_fix.tile([T, 1], F32, tag="nmx")
                nc.vector.reduce_max(nmx[:], scr[:], axis=mybir.AxisListType.X, negate=True)
                ex = att_fix.tile([T, n_steps], F32, tag="ex")
                sm = att_fix.tile([T, 1], F32, tag="sm")
                nc.scalar.activation(ex[:], scr[:], AF.Exp, bias=nmx[:], accum_out=sm[:])
                rc = att_fix.tile([T, 1], F32, tag="rc")
                nc.vector.reciprocal(rc[:], sm[:])
                cof = att_fix.tile([T, n_steps], BF16, tag="cof")
                nc.vector.tensor_scalar(cof[:], ex[:], rc[:], None, op0=OP.mult)
                # coeff.T [n, 24] via PE transposes
                cT = att_fix.tile([128, nac * T], BF16, tag="cT")
                for j in range(nac):
                    tp = tp_ps_pool.tile([128, 128], BF16, tag="tp")
                    nc.tensor.transpose(tp[:, 0:T], cof[:, bass.ts(j, 128)], id_sb[0:T, 0:T])
                    nc.scalar.copy(cT[:, bass.ts(j, T)], tp[:, 0:T])
                # CL_b n-partitioned tiles via PE transposes
                clb = att_fix.tile([128, nac * KH * 128], BF16, tag="clb")
                for j in range(nac):
                    for k in range(KH):
                        tpc = tp_ps_pool.tile([128, 128], BF16, tag="tp")
                        nc.tensor.transpose(tpc[:], cl3[:, j * 128:(j + 1) * 128, k, b],
                                            id_sb[:, :])
                        nc.scalar.copy(clb[:, bass.ts(j * KH + k, 128)], tpc[:])
                # ctx.T [512, 24]
                ctxp = ctx_ps_pool.tile([128, KH * T], F32, tag="ctxp")
                for k in range(KH):
                    for j in range(nac):
                        nc.tensor.matmul(ctxp[:, bass.ts(k, T)],
                                         clb[:, bass.ts(j * KH + k, 128)],
                                         cT[:, bass.ts(j, T)],
                                         start=(j == 0), stop=(j == nac - 1))
                # out [24, 32]
                ob_ps = scr_ps_pool.tile([T, C], F32, tag="ob_ps")
                for jj in range(8):
                    lr = att_fix.tile([128, T], BF16, tag="lr")
                    src = hd_v[:, jj, :, b] if jj < KH else ctxp[:, bass.ts(jj - KH, T)]
                    nc.scalar.activation(lr[:], src, AF.Lrelu, alpha=0.01)
                    nc.tensor.matmul(ob_ps[:], lr[:], wout_sb[:, bass.ts(jj, C)],
                                     start=(jj == 0), stop=(jj == 7))
                ob = att_fix.tile([T, C], F32, tag="ob")
                nc.vector.tensor_tensor(ob[:], ob_ps[:], bout_sb[:], op=OP.add)
                nc.sync.dma_start(p_out[b], ob[:])
    nc.compile()
    return nc


def prep_inputs(condition, Wih_enc, Whh_enc, bih_enc, bhh_enc,
                Wih_cell, Whh_cell, bih_cell, bhh_cell, W_out, b_out,
                n_steps=N_STEPS):
    def tile64(Wt):  # [512, 2048] -> [128, 64*128], col block m*4+k
        return np.ascontiguousarray(
            Wt.reshape(KH, 128, M16, 128).transpose(1, 2, 0, 3).reshape(128, 64 * 128)).astype(BF)

    # encoder Whh: reordered tiles [i i f f g g o o] per half, g-rows scaled x2
    WhhT = Whh_enc.T.astype(np.float32).copy()        # [512, 2048]
    WhhT[:, 1024:1536] *= 2.0                          # g-gate columns
    t4 = WhhT.reshape(KH, 128, M16, 128)               # [k, p, m, col]
    whh_new = np.empty((128, 64 * 128), np.float32)
    for mp, mo in enumerate(MPRIME):
        for k in range(KH):
            whh_new[:, (mp * KH + k) * 128:(mp * KH + k + 1) * 128] = t4[k, :, mo, :]
    # encoder Wih augmented with bias row, same permutation + g scaling
    wih_a = np.zeros((128, G), np.float32)
    wih_a[0:C] = Wih_enc.T
    wih_a[C] = bih_enc + bhh_enc
    wih_a[:, 1024:1536] *= 2.0
    wih_new = np.empty_like(wih_a)
    for mp, mo in enumerate(MPRIME):
        wih_new[:, mp * 128:(mp + 1) * 128] = wih_a[:, mo * 128:(mo + 1) * 128]

    wcellT = tile64(Wih_cell.T.astype(np.float32))
    bias_c = (bih_cell + bhh_cell).astype(np.float32)
    bcell_bc = np.repeat(bias_c.reshape(M16, 128).T[:, :, None], B, axis=2).reshape(128, 256)
    woutT = np.ascontiguousarray(
        W_out.T.reshape(8, 128, C).transpose(1, 0, 2).reshape(128, 8 * C)).astype(BF)
    bout_bc = np.tile(b_out[None, :].astype(np.float32), (T, 1))
    ident = np.eye(128, dtype=np.float32).astype(BF)

    shared = {
        "wihT_a": wih_new.astype(BF), "whhT": whh_new.astype(BF), "wcellT": wcellT,
        "bcell_bc": bcell_bc.astype(np.float32), "woutT": woutT,
        "bout_bc": bout_bc, "ident": ident,
    }
    maps = []
    for core in range(NCORES):
        cb = condition[core * B:(core + 1) * B, :n_steps, :]  # [16, n, 32]
        ca = np.zeros((128, n_steps * B), np.float32)
        ca[C] = 1.0
        ca[0:C] = cb.transpose(2, 1, 0).reshape(C, n_steps * B)  # col = n*16+b
        m = dict(shared)
        m["cond_aT"] = ca.astype(BF)
        maps.append(m)
    return maps


_NC_CACHE = {}
LAST_RESULT = None


def _ensure_ntff_hook():
    """The agent image's antenv lacks axon_hooks; provide it and register the
    ctypes NTFF profiling hook so trace=True works under axon."""
    import types
    if "antenv.axon_hooks" in sys.modules:
        return
    mod = types.ModuleType("antenv.axon_hooks")
    _h = [None]
    mod.set_axon_ntff_profile_hook = lambda h: _h.__setitem__(0, h)
    mod.get_axon_ntff_profile_hook = lambda: _h[0]
    sys.modules["antenv.axon_hooks"] = mod
    if "/root/.axon_site" not in sys.path:
        sys.path.insert(0, "/root/.axon_site")
    from trn_agent_boot.trn_boot import _ntff_profile_via_ctypes
    mod.set_axon_ntff_profile_hook(_ntff_profile_via_ctypes("/opt/axon/libaxon_pjrt.so"))


def kernel(_trace=False, **inputs):
    global LAST_RESULT
    if _trace:
        try:
            _ensure_ntff_hook()
        except Exception as e:
            print("ntff hook setup failed:", e)
    inputs = {k: np.asarray(v) for k, v in inputs.items()}
    n_steps = N_STEPS
    if n_steps not in _NC_CACHE:
        _NC_CACHE[n_steps] = build_program(n_steps)
    nc = _NC_CACHE[n_steps]
    maps = prep_inputs(**inputs, n_steps=n_steps)
    res = run_bass_kernel_spmd(nc, maps, list(range(NCORES)), trace=_trace)
    LAST_RESULT = res
    out = np.concatenate([np.asarray(res.results[i]["out"], dtype=np.float32)
                          for i in range(NCORES)], axis=0)
    return out


# revision 17
# speedup vs baseline: 1.0591x; 1.0591x over previous
"""Trainium2 Bass kernel for nn_CCGGenerator (LSTM encoder + attention decoder).

Sharding: data-parallel, batch 128 -> 16 per core across 8 cores.
All weights replicated. Self-contained; everything hardcoded.

Per-core design (B=16):
- Encoder gates computed transposed: gates.T [2048, 16] as 16 PSUM tiles
  [128, 16] packed in one [128, 256] region. Weight tiles reordered as
  [i0 i1 f0 f1 g0 g1 o0 o1 | i2 i3 f2 f3 g2 g3 o2 o3] so each half of the
  hidden dim (A = h-dims 0..255, B = 256..511) is a contiguous 128-col block.
- xg (input projection + bias) is FUSED into the same PSUM accumulation as
  16 extra K=33 matmuls (start=True), eliminating the per-step bias adds.
- tanh(g) = 2*sigmoid(2g) - 1 with g-rows of the weights pre-scaled by 2 ->
  ONE sigmoid over each 128-col half instead of 3 activations.
- Tail split into halves A/B whose ops pipeline against the next step's
  matmuls (k-ordered so MMs needing hA run first); LDWEIGHTS fully overlaps
  MMs on the PE so the matmul burst is ~27ns/MM.
- The whole condition-latent CL lives in SBUF: cl_sb [128, N*64] bf16 with
  col = n*64 + k*16 + b  (k = hidden 128-chunk). No DMA in the recurrence.
- Decoder: 24 steps into hd_sb [128, 4*24*16]; attention per b as before.
"""
import sys
sys.path.insert(0, "/opt/trn_rl_repo")

import numpy as np
import ml_dtypes
from contextlib import ExitStack

import concourse.bass as bass
import concourse.tile as tile
from concourse import bacc, mybir
from concourse.bass_utils import run_bass_kernel_spmd

F32 = mybir.dt.float32
BF16 = mybir.dt.bfloat16
AF = mybir.ActivationFunctionType
OP = mybir.AluOpType
BF = ml_dtypes.bfloat16

NCORES = 8
B = 16          # batch per core
N_STEPS = 1024  # encoder sequence length
SCH_C = 64      # steps per cond chunk
C = 32
H = 512
G = 2048        # 4H
T = 24
KH = 4          # hidden 128-chunks
M16 = 16        # gate-dim 128-chunks

# m' tile order: halves of the hidden dim, [i i f f g g o o] within a half
MPRIME = [0, 1, 4, 5, 8, 9, 12, 13, 2, 3, 6, 7, 10, 11, 14, 15]


def build_program(n_steps=N_STEPS):
    nac = n_steps // 128  # attention n-chunks
    nc = bacc.Bacc("TRN2", target_bir_lowering=False, debug=False,
                   num_devices=NCORES)

    p_cond = nc.declare_dram_parameter("cond_aT", [128, n_steps * B], BF16, isOutput=False)
    p_wih = nc.declare_dram_parameter("wihT_a", [128, G], BF16, isOutput=False)
    p_whh = nc.declare_dram_parameter("whhT", [128, 64 * 128], BF16, isOutput=False)
    p_wcell = nc.declare_dram_parameter("wcellT", [128, 64 * 128], BF16, isOutput=False)
    p_bcell = nc.declare_dram_parameter("bcell_bc", [128, 256], F32, isOutput=False)
    p_wout = nc.declare_dram_parameter("woutT", [128, 8 * C], BF16, isOutput=False)
    p_bout = nc.declare_dram_parameter("bout_bc", [T, C], F32, isOutput=False)
    p_id = nc.declare_dram_parameter("ident", [128, 128], BF16, isOutput=False)
    p_out = nc.declare_dram_parameter("out", [B, T, C], F32, isOutput=True)

    with tile.TileContext(nc) as tc, ExitStack() as ctx:
        const = ctx.enter_context(tc.tile_pool(name="const", bufs=1))

        cl_sb = const.tile([128, n_steps * KH * B], BF16, tag="cl_sb")  # 16 MB
        hd_sb = const.tile([128, KH * T * B], BF16, tag="hd_sb")
        wout_sb = const.tile([128, 8 * C], BF16, tag="wout_sb")
        nc.sync.dma_start(wout_sb[:], p_wout[:])
        bout_sb = const.tile([T, C], F32, tag="bout_sb")
        nc.sync.dma_start(bout_sb[:], p_bout[:])
        id_sb = const.tile([128, 128], BF16, tag="id_sb")
        nc.sync.dma_start(id_sb[:], p_id[:])
        bcell_sb = const.tile([128, 256], F32, tag="bcell_sb")
        nc.sync.dma_start(bcell_sb[:], p_bcell[:])

        cl3 = cl_sb[:].rearrange("p (n k b) -> p n k b", k=KH, b=B)
        hd_v = hd_sb[:].rearrange("p (k t b) -> p k t b", k=KH, t=T)

        # ---------------- encoder ----------------
        # Two PSUM partials in separate banks (full-bank [128,512] tiles):
        #   Ge = X + k0 + k1 per tile (gated by hA of prev step)
        #   Gl = k2 + k3 per tile     (gated by hB of prev step)
        # Within a bank each tile's accumulation group is contiguous
        # (start=True clears has_written for the whole bank).  The early
        # partial is copied to SBUF during the gap, then one DVE add
        # (SBUF + PSUM) feeds the sigmoid.  Step parity rotates which half
        # (A = h-dims 0..255 / B) finishes first, so the late-half penalty
        # alternates.
        with tc.tile_pool(name="enc", bufs=1) as enc_pool, \
             tc.tile_pool(name="cond", bufs=2) as cond_pool, \
             tc.tile_pool(name="ga", bufs=2) as ga_pool, \
             tc.tile_pool(name="st", bufs=2) as st_pool, \
             tc.tile_pool(name="gps", bufs=1, space="PSUM") as gps_pool:
            whh_sb = enc_pool.tile([128, 64 * 128], BF16, tag="whh_sb")
            nc.sync.dma_start(whh_sb[:], p_whh[:])
            wih_sb = enc_pool.tile([128, G], BF16, tag="wih_sb")
            nc.sync.dma_start(wih_sb[:], p_wih[:])
            c_f = enc_pool.tile([128, KH * B], F32, tag="c_f")
            h0 = enc_pool.tile([128, KH * B], BF16, tag="h0")
            nc.any.memset(c_f[:], 0.0)
            nc.any.memset(h0[:], 0.0)

            cond_ch = None
            for n in range(n_steps):
                if n % SCH_C == 0:
                    cond_ch = cond_pool.tile([128, SCH_C * B], BF16, tag="cond_ch")
                    nc.sync.dma_start(cond_ch[:], p_cond[:, bass.ts(n // SCH_C, SCH_C * B)])
                soff = (n % SCH_C) * B
                if n == 0:
                    hin = lambda k: h0[:, bass.ts(k, B)]
                else:
                    hin = lambda k, _n=n: cl_sb[:, (_n - 1) * 64 + k * B:(_n - 1) * 64 + (k + 1) * B]

                par = n % 2          # 0: A first; 1: B first
                halves = (0, 1) if par == 0 else (1, 0)
                # PSUM partials, one full bank each (single-buffered; WAR deps
                # on their readers sequence the steps):
                #   gx  [X per tile, single-MM groups] - prefills during the
                #       previous tail (gated only by cond / cX copy)
                #   gke [ke0+ke1 per tile, per half]   - gated by early h half
                #   gkl [kl0+kl1 per tile, per half]   - gated by late h half
                gx = gps_pool.tile([128, 512], F32, tag="gx", name="gx")
                gke = [None, None]
                gkl = [None, None]
                for hf in (0, 1):
                    gke[hf] = gps_pool.tile([128, 512], F32, tag=f"gke{hf}", name=f"gke{hf}")
                    gkl[hf] = gps_pool.tile([128, 512], F32, tag=f"gkl{hf}", name=f"gkl{hf}")
                # ke: chunks of the h half that step n-1 finished first (the
                # opposite parity's first half); kl: the other two chunks.
                ke = (2, 3) if par == 0 else (0, 1)
                kl = (0, 1) if par == 0 else (2, 3)
                for m in range(M16):
                    nc.tensor.matmul(gx[:, bass.ts(m, B)], wih_sb[:, bass.ts(m, 128)],
                                     cond_ch[:, soff:soff + B], start=True, stop=True)
                # cX: bulk copy of the X partial (ACT, early in the step)
                cX = ga_pool.tile([128, 256], F32, tag="cX", name="cX")
                nc.scalar.copy(cX[:], gx[:, 0:256])
                # MM phases interleaved with their consumers (s1 = cX + gke off
                # the critical path; ga = s1 + gkl feeds the sigmoid)
                s1 = [None, None]
                ga = [None, None]
                for hf in halves:
                    for j in range(8):
                        m = hf * 8 + j
                        nc.tensor.matmul(gke[hf][:, bass.ts(j, B)],
                                         whh_sb[:, bass.ts(m * KH + ke[0], 128)],
                                         hin(ke[0]), start=True, stop=False)
                        nc.tensor.matmul(gke[hf][:, bass.ts(j, B)],
                                         whh_sb[:, bass.ts(m * KH + ke[1], 128)],
                                         hin(ke[1]), start=False, stop=True)
                    s1[hf] = ga_pool.tile([128, 128], F32, tag=f"s1_{hf}", name=f"s1_{hf}")
                    nc.vector.tensor_tensor(s1[hf][:], cX[:, hf * 128:hf * 128 + 128],
                                            gke[hf][:, 0:128], op=OP.add)
                    for j in range(8):
                        m = hf * 8 + j
                        nc.tensor.matmul(gkl[hf][:, bass.ts(j, B)],
                                         whh_sb[:, bass.ts(m * KH + kl[0], 128)],
                                         hin(kl[0]), start=True, stop=False)
                        nc.tensor.matmul(gkl[hf][:, bass.ts(j, B)],
                                         whh_sb[:, bass.ts(m * KH + kl[1], 128)],
                                         hin(kl[1]), start=False, stop=True)
                    ga[hf] = ga_pool.tile([128, 128], F32, tag=f"ga{hf}", name=f"ga{hf}")
                    nc.vector.tensor_tensor(ga[hf][:], s1[hf][:], gkl[hf][:, 0:128], op=OP.add)
                # tails: the early half's FULL chain first, then the late half's
                for hf in halves:
                    hs = hf * 32
                    nc.scalar.activation(ga[hf][:], ga[hf][:], AF.Sigmoid)
                    g_sl = ga[hf][:, 64:96]
                    nc.vector.tensor_scalar(g_sl, g_sl, 2.0, -1.0, op0=OP.mult, op1=OP.add)
                    c2 = st_pool.tile([128, 32], F32, tag=f"c2{hf}", name="c2")
                    nc.vector.tensor_tensor(c2[:], ga[hf][:, 32:64], c_f[:, hs:hs + 32], op=OP.mult)
                    ig = st_pool.tile([128, 32], F32, tag=f"ig{hf}", name="ig")
                    nc.vector.tensor_tensor(ig[:], ga[hf][:, 0:32], g_sl, op=OP.mult)
                    nc.vector.tensor_tensor(c_f[:, hs:hs + 32], c2[:], ig[:], op=OP.add)
                    th = st_pool.tile([128, 32], F32, tag=f"th{hf}", name="th")
                    nc.scalar.activation(th[:], c_f[:, hs:hs + 32], AF.Tanh)
                    nc.vector.tensor_tensor(cl_sb[:, n * 64 + hs:n * 64 + hs + 32],
                                            ga[hf][:, 96:128], th[:], op=OP.mult)

        # ---------------- decoder (baseline structure) ----------------
        with tc.tile_pool(name="dec", bufs=1) as dec_pool, \
             tc.tile_pool(name="dg_ps", bufs=2, space="PSUM") as dg_ps_pool, \
             tc.tile_pool(name="dgtmp", bufs=2) as dgtmp_pool, \
             tc.tile_pool(name="dst", bufs=2) as dst_pool:
            wcell_sb = dec_pool.tile([128, 64 * 128], BF16, tag="wcell_sb")
            nc.sync.dma_start(wcell_sb[:], p_wcell[:])
            for t in range(T):
                if t == 0:
                    hin = lambda k: cl_sb[:, (n_steps - 1) * 64 + k * B:(n_steps - 1) * 64 + (k + 1) * B]
                else:
                    hin = lambda k, _t=t: hd_sb[:, k * T * B + (_t - 1) * B:k * T * B + _t * B]
                gps = dg_ps_pool.tile([128, 256], F32, tag="gps")
                for m in range(M16):
                    for k in range(KH):
                        nc.tensor.matmul(gps[:, bass.ts(m, B)],
                                         wcell_sb[:, bass.ts(m * KH + k, 128)],
                                         hin(k), start=(k == 0), stop=(k == KH - 1))
                ga = dgtmp_pool.tile([128, 256], F32, tag="ga")
                nc.vector.tensor_tensor(ga[:], gps[:], bcell_sb[:], op=OP.add)
                nc.scalar.activation(ga[:, 0:128], ga[:, 0:128], AF.Sigmoid)     # i, f
                nc.scalar.activation(ga[:, 128:192], ga[:, 128:192], AF.Tanh)    # g
                nc.scalar.activation(ga[:, 192:256], ga[:, 192:256], AF.Sigmoid)  # o
                cd = dst_pool.tile([128, 64], F32, tag="cd")
                nc.vector.tensor_tensor(cd[:], ga[:, 0:64], ga[:, 128:192], op=OP.mult)
                th = dst_pool.tile([128, 64], F32, tag="th")
                nc.scalar.activation(th[:], cd[:], AF.Tanh)
                for k in range(KH):
                    nc.vector.tensor_tensor(hd_sb[:, k * T * B + t * B:k * T * B + (t + 1) * B],
                                            ga[:, 192 + k * B:192 + (k + 1) * B],
                                            th[:, bass.ts(k, B)], op=OP.mult)

        # ---------------- attention + output, per batch ----------------
        with tc.tile_pool(name="att_fix", bufs=2) as att_fix, \
             tc.tile_pool(name="scr_ps", bufs=1, space="PSUM") as scr_ps_pool, \
             tc.tile_pool(name="tp_ps", bufs=2, space="PSUM") as tp_ps_pool, \
             tc.tile_pool(name="ctx_ps", bufs=2, space="PSUM") as ctx_ps_pool:
            for b in range(B):
                # scores [24, n]: lhsT = hd strided, rhs = cl_sb strided (CL.T native)
                scr = scr_ps_pool.tile([T, n_steps], F32, tag="scr")
                scn = min(512, n_steps)
                for k in range(KH):
                    for j in range(n_steps // scn):
                        rhs = cl3[:, j * scn:(j + 1) * scn, k, b]
                        nc.tensor.matmul(scr[:, bass.ts(j, scn)], hd_v[:, k, :, b],
                                         rhs, start=(k == 0), stop=(k == KH - 1))
                nmx = att_fix.tile([T, 1], F32, tag="nmx")
                nc.vector.reduce_max(nmx[:], scr[:], axis=mybir.AxisListType.X, negate=True)
                ex = att_fix.tile([T, n_steps], F32, tag="ex")
                sm = att_fix.tile([T, 1], F32, tag="sm")
                nc.scalar.activation(ex[:], scr[:], AF.Exp, bias=nmx[:], accum_out=sm[:])
                rc = att_fix.tile([T, 1], F32, tag="rc")
                nc.vector.reciprocal(rc[:], sm[:])
                cof = att_fix.tile([T, n_steps], BF16, tag="cof")
                nc.vector.tensor_scalar(cof[:], ex[:], rc[:], None, op0=OP.mult)
                # coeff.T [n, 24] via PE transposes
                cT = att_fix.tile([128, nac * T], BF16, tag="cT")
                for j in range(nac):
                    tp = tp_ps_pool.tile([128, 128], BF16, tag="tp")
                    nc.tensor.transpose(tp[:, 0:T], cof[:, bass.ts(j, 128)], id_sb[0:T, 0:T])
                    nc.scalar.copy(cT[:, bass.ts(j, T)], tp[:, 0:T])
                # CL_b n-partitioned tiles via PE transposes
                clb = att_fix.tile([128, nac * KH * 128], BF16, tag="clb")
                for j in range(nac):
                    for k in range(KH):
                        tpc = tp_ps_pool.tile([128, 128], BF16, tag="tp")
                        nc.tensor.transpose(tpc[:], cl3[:, j * 128:(j + 1) * 128, k, b],
                                            id_sb[:, :])
                        nc.scalar.copy(clb[:, bass.ts(j * KH + k, 128)], tpc[:])
                # ctx.T [512, 24]
                ctxp = ctx_ps_pool.tile([128, KH * T], F32, tag="ctxp")
                for k in range(KH):
                    for j in range(nac):
                        nc.tensor.matmul(ctxp[:, bass.ts(k, T)],
                                         clb[:, bass.ts(j * KH + k, 128)],
                                         cT[:, bass.ts(j, T)],
                                         start=(j == 0), stop=(j == nac - 1))
                # out [24, 32]
                ob_ps = scr_ps_pool.tile([T, C], F32, tag="ob_ps")
                for jj in range(8):
                    lr = att_fix.tile([128, T], BF16, tag="lr")
                    src = hd_v[:, jj, :, b] if jj < KH else ctxp[:, bass.ts(jj - KH, T)]
                    nc.scalar.activation(lr[:], src, AF.Lrelu, alpha=0.01)
                    nc.tensor.matmul(ob_ps[:], lr[:], wout_sb[:, bass.ts(jj, C)],
                                     start=(jj == 0), stop=(jj == 7))
                ob = att_fix.tile([T, C], F32, tag="ob")
                nc.vector.tensor_tensor(ob[:], ob_ps[:], bout_sb[:], op=OP.add)
                nc.sync.dma_start(p_out[b], ob[:])
    nc.compile()
    return nc


def prep_inputs(condition, Wih_enc, Whh_enc, bih_enc, bhh_enc,
                Wih_cell, Whh_cell, bih_cell, bhh_cell, W_out, b_out,
                n_steps=N_STEPS):
    def tile64(Wt):  # [512, 2048] -> [128, 64*128], col block m*4+k
        return np.ascontiguousarray(
            Wt.reshape(KH, 128, M16, 128).transpose(1, 2, 0, 3).reshape(128, 64 * 128)).astype(BF)

    # encoder Whh: reordered tiles [i i f f g g o o] per half, g-rows scaled x2
    WhhT = Whh_enc.T.astype(np.float32).copy()        # [512, 2048]
    WhhT[:, 1024:1536] *= 2.0                          # g-gate columns
    t4 = WhhT.reshape(KH, 128, M16, 128)               # [k, p, m, col]
    whh_new = np.empty((128, 64 * 128), np.float32)
    for mp, mo in enumerate(MPRIME):
        for k in range(KH):
            whh_new[:, (mp * KH + k) * 128:(mp * KH + k + 1) * 128] = t4[k, :, mo, :]
    # encoder Wih augmented with bias row, same permutation + g scaling
    wih_a = np.zeros((128, G), np.float32)
    wih_a[0:C] = Wih_enc.T
    wih_a[C] = bih_enc + bhh_enc
    wih_a[:, 1024:1536] *= 2.0
    wih_new = np.empty_like(wih_a)
    for mp, mo in enumerate(MPRIME):
        wih_new[:, mp * 128:(mp + 1) * 128] = wih_a[:, mo * 128:(mo + 1) * 128]

    wcellT = tile64(Wih_cell.T.astype(np.float32))
    bias_c = (bih_cell + bhh_cell).astype(np.float32)
    bcell_bc = np.repeat(bias_c.reshape(M16, 128).T[:, :, None], B, axis=2).reshape(128, 256)
    woutT = np.ascontiguousarray(
        W_out.T.reshape(8, 128, C).transpose(1, 0, 2).reshape(128, 8 * C)).astype(BF)
    bout_bc = np.tile(b_out[None, :].astype(np.float32), (T, 1))
    ident = np.eye(128, dtype=np.float32).astype(BF)

    shared = {
        "wihT_a": wih_new.astype(BF), "whhT": whh_new.astype(BF), "wcellT": wcellT,
        "bcell_bc": bcell_bc.astype(np.float32), "woutT": woutT,
        "bout_bc": bout_bc, "ident": ident,
    }
    maps = []
    for core in range(NCORES):
        cb = condition[core * B:(core + 1) * B, :n_steps, :]  # [16, n, 32]
        ca = np.zeros((128, n_steps * B), np.float32)
        ca[C] = 1.0
        ca[0:C] = cb.transpose(2, 1, 0).reshape(C, n_steps * B)  # col = n*16+b
        m = dict(shared)
        m["cond_aT"] = ca.astype(BF)
        maps.append(m)
    return maps


_NC_CACHE = {}
LAST_RESULT = None


def _ensure_ntff_hook():
    """The agent image's antenv lacks axon_hooks; provide it and register the
    ctypes NTFF profiling hook so trace=True works under axon."""
    import types
    if "antenv.axon_hooks" in sys.modules:
        return
    mod = types.ModuleType("antenv.axon_hooks")
    _h = [None]
    mod.set_axon_ntff_profile_hook = lambda h: _h.__setitem__(0, h)
    mod.get_axon_ntff_profile_hook = lambda: _h[0]
    sys.modules["antenv.axon_hooks"] = mod
    if "/root/.axon_site" not in sys.path:
        sys.path.insert(0, "/root/.axon_site")
    from trn_agent_boot.trn_boot import _ntff_profile_via_ctypes
    mod.set_axon_ntff_profile_hook(_ntff_profile_via_ctypes("/opt/axon/libaxon_pjrt.so"))


def kernel(_trace=False, **inputs):
    global LAST_RESULT
    if _trace:
        try:
            _ensure_ntff_hook()
        except Exception as e:
            print("ntff hook setup failed:", e)
    inputs = {k: np.asarray(v) for k, v in inputs.items()}
    n_steps = N_STEPS
    if n_steps not in _NC_CACHE:
        _NC_CACHE[n_steps] = build_program(n_steps)
    nc = _NC_CACHE[n_steps]
    maps = prep_inputs(**inputs, n_steps=n_steps)
    res = run_bass_kernel_spmd(nc, maps, list(range(NCORES)), trace=_trace)
    LAST_RESULT = res
    out = np.concatenate([np.asarray(res.results[i]["out"], dtype=np.float32)
                          for i in range(NCORES)], axis=0)
    return out
